# revision 13
# baseline (speedup 1.0000x reference)
"""AngleLinear (A-Softmax margin loss forward) on 8 Trainium2 NeuronCores.

Math (reference, with x:[N,D], target:[N], weight:[D,C]):
    w_hat   = weight / ||weight||_col
    cos     = clip((x @ w_hat) / ||x||_row / ||w_hat||_col, -1, 1)   # [N, C]
    out     = cos * ||x||_row
    out[n, target[n]] += (phi(c_t) - c_t) * ||x|| / (1 + lambda)

Facts used (validated against the reference on the actual input data):
  * ||w_hat||_col == 1 up to f32 roundoff, so away from target positions
    out == x @ w_hat.
  * |cos| < 0.25 for this data, so the clip to [-1,1] never binds on the
    bulk path (c_t itself is still clipped).
  * phi needs no arccos: k = floor(4*arccos(c)/pi) equals
    (c<=cos(pi/4)) + (c<=cos(pi/2)) + (c<=cos(3pi/4)), and
    phi = (1-2*(k mod 2))*(8c^4-8c^2+1) - 2k.

Sharding: tensor-parallel over the class dimension C. Each of the 8 cores
owns a 12500-column slice of w_hat and produces the matching slice of the
output; no collectives are needed. Host staging normalizes the weight
columns in f32 (exactly as the reference does) and casts to bf16 — an
O(D*C) per-element transform of the same class as the dtype cast. All
O(N*C*D) math runs on device.

The target scatter is a local masked update (per the sharding hint):
core m receives tcol[n] = target[n] - m*12500 - ci*500 per column tile and
adds `addition[n]` where iota==tcol; rows whose target falls in another
shard never match. The margin path (c_t via gathered target columns, phi,
k, addition) is computed on-device by every core.

Per-core engine placement (approx busy):
  PE  : 4x(K=128) bf16 matmuls per [128,500] psum tile (~92us) + tiny
        column sums for the c_t path.
  DVE : psum+corr -> sbuf evict (f32, ~59us), iota==tcol correction build
        (16-bit, 2x mode, ~35us), small vector math.
  ACT : x^2 square, sqrt.
  DMA : w_hat shard in (bf16, 12.5MB), out shard (bf16, 12.5MB).
Output tiles are staged bf16 on device and upcast to f32 on gather (the
bf16 matmul already bounds precision; measured rel err ~4e-3 << 2e-2).
"""

import sys
from contextlib import ExitStack

for _p in ("/opt/trn_rl_repo",):
    if _p not in sys.path:
        sys.path.append(_p)

import numpy as np
import ml_dtypes

from concourse import bacc, mybir, tile
from concourse.bass_utils import run_bass_kernel_spmd

BF16 = mybir.dt.bfloat16
F16 = mybir.dt.float16
F32 = mybir.dt.float32
ALU = mybir.AluOpType
AF = mybir.ActivationFunctionType

# problem constants (hardcoded; kernel.py must be self-contained)
N = 512
D = 512
C = 100000
NCORES = 8
CS = C // NCORES  # 12500 columns per core
KI = D // 128  # 4 contraction chunks
MI = N // 128  # 4 output row chunks
CTILE = 500  # matmul free dim (one PSUM bank)
CW = 1000  # dma/compute super-tile width
NW = (CS + CW - 1) // CW  # 13 super-tiles (12 full + one 500 tail)

PI = 3.141592653  # matches the reference source
IT = 1
CUR_LAMBDA = max(5.0, 1500.0 / (1.0 + 0.1 * IT))
INV1PL = float(np.float32(1.0) / np.float32(1.0 + CUR_LAMBDA))
T1 = float(np.float32(np.cos(PI / 4)))
T2 = float(np.float32(np.cos(PI / 2)))
T3 = float(np.float32(np.cos(3 * PI / 4)))

OUT_DT = BF16  # on-device output staging dtype (upcast to f32 on gather)

_CACHE = {}


def _build():
    nc = bacc.Bacc("TRN2", target_bir_lowering=False, debug=False, num_devices=NCORES)

    xt_d = nc.dram_tensor("xt", [D, N], BF16, kind="ExternalInput").ap()
    wt_d = nc.dram_tensor("wt", [D, N], BF16, kind="ExternalInput").ap()
    w_d = nc.dram_tensor("w", [D, CS], BF16, kind="ExternalInput").ap()
    iota_d = nc.dram_tensor("iota", [128, CW], F16, kind="ExternalInput").ap()
    tcol_d = nc.dram_tensor("tcol", [128, MI, NW], F32, kind="ExternalInput").ap()
    out_d = nc.dram_tensor("out", [N, CS], OUT_DT, kind="ExternalOutput").ap()
    scratch_d = nc.dram_tensor("scratch", [2 * N], F32).ap()

    with tile.TileContext(nc) as tc, ExitStack() as ctx:
        consts = ctx.enter_context(tc.tile_pool(name="consts", bufs=1))
        prep = ctx.enter_context(tc.tile_pool(name="prep", bufs=1))
        small = ctx.enter_context(tc.tile_pool(name="small", bufs=1))
        wpool = ctx.enter_context(tc.tile_pool(name="wpool", bufs=3))
        corrpool = ctx.enter_context(tc.tile_pool(name="corrpool", bufs=4))
        outpool = ctx.enter_context(tc.tile_pool(name="outpool", bufs=3))
        pspool = ctx.enter_context(tc.tile_pool(name="pspool", bufs=6, space="PSUM"))
        cspool = ctx.enter_context(tc.tile_pool(name="cspool", bufs=2, space="PSUM"))

        # ---- resident constants ------------------------------------------------
        xt_sb = consts.tile([128, KI, N], BF16)
        nc.sync.dma_start(out=xt_sb[:], in_=xt_d.rearrange("(k p) n -> p k n", p=128))
        wt_sb = consts.tile([128, KI, N], BF16)
        nc.sync.dma_start(out=wt_sb[:], in_=wt_d.rearrange("(k p) n -> p k n", p=128))
        iota_sb = consts.tile([128, CW], F16)
        nc.sync.dma_start(out=iota_sb[:], in_=iota_d[:])
        tcol_sb = consts.tile([128, MI, NW], F32)
        nc.sync.dma_start(out=tcol_sb[:], in_=tcol_d[:])
        ones_bf = consts.tile([128, 1], BF16)
        nc.vector.memset(ones_bf[:], 1.0)

        # ---- c_t / margin path (tiny) ------------------------------------------
        # column sums of x*wt_hat and x^2 over D -> two [1, N] rows in PSUM
        prod = prep.tile([128, KI, N], BF16)
        xsq = prep.tile([128, KI, N], BF16)
        nc.vector.tensor_mul(prod[:], xt_sb[:], wt_sb[:])
        nc.scalar.square(xsq[:], xt_sb[:])

        pk = prep.tile([1, 2, N], F32)
        for j, src_t in enumerate((prod, xsq)):
            cs_ps = cspool.tile([1, N], F32, tag="cs", name=f"prep_cs_{j}")
            for k in range(KI):
                nc.tensor.matmul(
                    cs_ps[:], ones_bf[:], src_t[:, k, :], start=k == 0, stop=k == KI - 1
                )
            nc.vector.tensor_copy(pk[:, j, :], cs_ps[:])
        # reshape [1, 2, 512] -> [128, 2, 4]: row n = mi*128+p -> (p, mi)
        nc.sync.dma_start(out=scratch_d[:], in_=pk[:])
        vecs = consts.tile([128, 2, MI], F32)
        nc.sync.dma_start(
            out=vecs[:], in_=scratch_d.rearrange("(j m p) -> p j m", p=128, j=2)
        )
        crow_t = vecs[:, 0, :]
        xnsq_t = vecs[:, 1, :]

        xn = small.tile([128, MI], F32)
        nc.scalar.sqrt(xn[:], xnsq_t)
        invxn = small.tile([128, MI], F32)
        nc.vector.reciprocal(invxn[:], xn[:])

        ct = small.tile([128, MI], F32)
        nc.vector.tensor_mul(ct[:], crow_t, invxn[:])
        nc.vector.tensor_scalar_min(ct[:], ct[:], 1.0)
        nc.vector.tensor_scalar_max(ct[:], ct[:], -1.0)

        c2 = small.tile([128, MI], F32)
        nc.vector.tensor_mul(c2[:], ct[:], ct[:])
        cosm = small.tile([128, MI], F32)
        nc.vector.tensor_mul(cosm[:], c2[:], c2[:])  # c^4
        nc.vector.tensor_sub(cosm[:], cosm[:], c2[:])  # c^4 - c^2
        nc.vector.tensor_scalar(cosm[:], cosm[:], 8.0, 1.0, op0=ALU.mult, op1=ALU.add)

        k1 = small.tile([128, MI], F32)
        nc.vector.tensor_single_scalar(k1[:], ct[:], T1, ALU.is_le)
        k2 = small.tile([128, MI], F32)
        nc.vector.tensor_single_scalar(k2[:], ct[:], T2, ALU.is_le)
        k3 = small.tile([128, MI], F32)
        nc.vector.tensor_single_scalar(k3[:], ct[:], T3, ALU.is_le)

        ksum = small.tile([128, MI], F32)
        nc.vector.tensor_add(ksum[:], k1[:], k2[:])
        nc.vector.tensor_add(ksum[:], ksum[:], k3[:])
        sgn = small.tile([128, MI], F32)
        nc.vector.tensor_sub(sgn[:], k1[:], k2[:])
        nc.vector.tensor_add(sgn[:], sgn[:], k3[:])  # k mod 2
        nc.vector.tensor_scalar(sgn[:], sgn[:], -2.0, 1.0, op0=ALU.mult, op1=ALU.add)

        phi = small.tile([128, MI], F32)
        nc.vector.tensor_mul(phi[:], sgn[:], cosm[:])
        nc.vector.tensor_scalar(ksum[:], ksum[:], 2.0, 0.0, op0=ALU.mult, op1=ALU.add)
        nc.vector.tensor_sub(phi[:], phi[:], ksum[:])  # phi = sign*cos_m - 2k

        addn_t = consts.tile([128, MI], F32)
        nc.vector.tensor_sub(addn_t[:], phi[:], ct[:])
        nc.vector.tensor_mul(addn_t[:], addn_t[:], xn[:])
        nc.vector.tensor_scalar(addn_t[:], addn_t[:], INV1PL, 0.0, op0=ALU.mult, op1=ALU.add)

        # ---- main loop over the class dimension --------------------------------
        w_r = w_d.rearrange("(k p) c -> p k c", p=128)
        for ci2 in range(NW):
            c_lo = ci2 * CW
            cw = min(CW, CS - c_lo)
            nh = cw // CTILE

            w_sb = wpool.tile([128, KI * cw], BF16, tag="w", name=f"w_{ci2}")
            nc.sync.dma_start(
                out=w_sb[:].rearrange("p (k c) -> p k c", k=KI),
                in_=w_r[:, :, c_lo : c_lo + cw],
            )

            for mi in range(MI):
                corr = corrpool.tile([128, CW], BF16, tag="corr", name=f"c_{ci2}_{mi}")
                nc.vector.tensor_scalar(
                    corr[:, :cw],
                    iota_sb[:, :cw],
                    tcol_sb[:, mi : mi + 1, ci2 : ci2 + 1],
                    addn_t[:, mi : mi + 1],
                    op0=ALU.is_equal,
                    op1=ALU.mult,
                )
                out_sb = outpool.tile(
                    [128, CW], OUT_DT, tag=f"out{mi}", name=f"o_{ci2}_{mi}"
                )
                for h in range(nh):
                    hs = slice(h * CTILE, (h + 1) * CTILE)
                    ps = pspool.tile([128, CTILE], F32, tag="ps", name=f"ps_{ci2}_{h}_{mi}")
                    for k in range(KI):
                        nc.tensor.matmul(
                            ps[:],
                            xt_sb[:, k, mi * 128 : (mi + 1) * 128],
                            w_sb[:, k * cw + h * CTILE : k * cw + (h + 1) * CTILE],
                            start=k == 0,
                            stop=k == KI - 1,
                        )
                    # PSUM -> SBUF eviction on the (otherwise idle) scalar engine
                    nc.scalar.activation(out_sb[:, hs], ps[:], AF.Copy)
                # correction: all-bf16 in-place add in SBUF (DVE 2x mode)
                nc.vector.tensor_add(out_sb[:, :cw], out_sb[:, :cw], corr[:, :cw])
                nc.sync.dma_start(
                    out=out_d[mi * 128 : (mi + 1) * 128, c_lo : c_lo + cw],
                    in_=out_sb[:, :cw],
                )

    nc.compile()
    return nc


def _get_nc():
    if "nc" not in _CACHE:
        _CACHE["nc"] = _build()
    return _CACHE["nc"]


def _prep_inputs(x, target, weight):
    x = np.asarray(x, dtype=np.float32)
    target = np.asarray(target).astype(np.int64)
    weight = np.asarray(weight, dtype=np.float32)

    # normalize columns in f32, exactly as the reference does, then cast bf16
    w_hat = weight / np.linalg.norm(weight, axis=0, keepdims=True)

    xt_bf = np.ascontiguousarray(x.T).astype(ml_dtypes.bfloat16)
    wt_bf = np.ascontiguousarray(w_hat[:, target]).astype(ml_dtypes.bfloat16)
    iota = np.broadcast_to(np.arange(CW, dtype=np.float16), (128, CW)).copy()

    in_maps = []
    for m in range(NCORES):
        w_bf = np.ascontiguousarray(w_hat[:, m * CS : (m + 1) * CS]).astype(
            ml_dtypes.bfloat16
        )
        # tcol[p, mi, ci2] = target[mi*128+p] - m*CS - ci2*CW, clamped to the
        # sentinel -1 when outside [0, CW) (never matches iota)
        tloc = (target - m * CS).reshape(MI, 128).T  # [128, MI] int64
        tcol = (
            tloc[:, :, None] - (np.arange(NW, dtype=np.int64) * CW)[None, None, :]
        )
        tcol = np.where((tcol >= 0) & (tcol < CW), tcol, -1)
        in_maps.append(
            {
                "xt": xt_bf,
                "wt": wt_bf,
                "w": w_bf,
                "iota": iota,
                "tcol": np.ascontiguousarray(tcol.astype(np.float32)),
            }
        )
    return in_maps


def kernel(x, target, weight, _trace=False, _trace_kwargs=None):
    nc = _get_nc()
    in_maps = _prep_inputs(x, target, weight)
    res = run_bass_kernel_spmd(
        nc,
        in_maps,
        core_ids=list(range(NCORES)),
        trace=_trace,
        **(_trace_kwargs or {}),
    )
    out = np.concatenate(
        [res.results[i]["out"].astype(np.float32) for i in range(NCORES)], axis=1
    )
    if _trace:
        _CACHE["last_result"] = res
    return out


if __name__ == "__main__":
    rng = np.random.default_rng(0)
    x = rng.standard_normal((N, D), dtype=np.float32)
    target = rng.integers(0, C, size=N)
    weight = rng.standard_normal((D, C), dtype=np.float32)
    out = kernel(x, target, weight)
    print("out", out.shape, out.dtype, float(np.abs(out).max()))


# revision 14
# speedup vs baseline: 1.1112x; 1.1112x over previous
"""AngleLinear (A-Softmax margin loss forward) on 8 Trainium2 NeuronCores.

Math (reference, with x:[N,D], target:[N], weight:[D,C]):
    w_hat   = weight / ||weight||_col
    cos     = clip((x @ w_hat) / ||x||_row / ||w_hat||_col, -1, 1)   # [N, C]
    out     = cos * ||x||_row
    out[n, target[n]] += (phi(c_t) - c_t) * ||x|| / (1 + lambda)

Facts used (validated against the reference on the actual input data):
  * ||w_hat||_col == 1 up to f32 roundoff, so away from target positions
    out == x @ w_hat.
  * |cos| < 0.25 for this data, so the clip to [-1,1] never binds on the
    bulk path (c_t itself is still clipped).
  * phi needs no arccos: k = floor(4*arccos(c)/pi) equals
    (c<=cos(pi/4)) + (c<=cos(pi/2)) + (c<=cos(3pi/4)), and
    phi = (1-2*(k mod 2))*(8c^4-8c^2+1) - 2k.

Sharding: tensor-parallel over the class dimension C. Each of the 8 cores
owns a 12500-column slice of w_hat and produces the matching slice of the
output; no collectives are needed. Host staging normalizes the weight
columns in f32 (exactly as the reference does) and casts to bf16 — an
O(D*C) per-element transform of the same class as the dtype cast. All
O(N*C*D) math runs on device.

The target scatter is a local masked update (per the sharding hint):
core m receives tcol[n] = target[n] - m*12500 - ci*500 per column tile and
adds `addition[n]` where iota==tcol; rows whose target falls in another
shard never match. The margin path (c_t via gathered target columns, phi,
k, addition) is computed on-device by every core.

Per-core engine placement (approx busy):
  PE  : 4x(K=128) bf16 matmuls per [128,500] psum tile (~92us) + tiny
        column sums for the c_t path.
  DVE : psum+corr -> sbuf evict (f32, ~59us), iota==tcol correction build
        (16-bit, 2x mode, ~35us), small vector math.
  ACT : x^2 square, sqrt.
  DMA : w_hat shard in (bf16, 12.5MB), out shard (bf16, 12.5MB).
Output tiles are staged bf16 on device and upcast to f32 on gather (the
bf16 matmul already bounds precision; measured rel err ~4e-3 << 2e-2).
"""

import sys
from contextlib import ExitStack

for _p in ("/opt/trn_rl_repo",):
    if _p not in sys.path:
        sys.path.append(_p)

import numpy as np
import ml_dtypes

from concourse import bacc, mybir, tile
from concourse.bass_utils import run_bass_kernel_spmd

BF16 = mybir.dt.bfloat16
F16 = mybir.dt.float16
F32 = mybir.dt.float32
ALU = mybir.AluOpType
AF = mybir.ActivationFunctionType

# problem constants (hardcoded; kernel.py must be self-contained)
N = 512
D = 512
C = 100000
NCORES = 8
CS = C // NCORES  # 12500 columns per core
KI = D // 128  # 4 contraction chunks
MI = N // 128  # 4 output row chunks
CTILE = 500  # matmul free dim (one PSUM bank)
CW = 1000  # dma/compute super-tile width
NW = (CS + CW - 1) // CW  # 13 super-tiles (12 full + one 500 tail)

PI = 3.141592653  # matches the reference source
IT = 1
CUR_LAMBDA = max(5.0, 1500.0 / (1.0 + 0.1 * IT))
INV1PL = float(np.float32(1.0) / np.float32(1.0 + CUR_LAMBDA))
T1 = float(np.float32(np.cos(PI / 4)))
T2 = float(np.float32(np.cos(PI / 2)))
T3 = float(np.float32(np.cos(3 * PI / 4)))

OUT_DT = BF16  # on-device output staging dtype (upcast to f32 on gather)

_CACHE = {}


def _build():
    nc = bacc.Bacc("TRN2", target_bir_lowering=False, debug=False, num_devices=NCORES)

    xt_d = nc.dram_tensor("xt", [D, N], BF16, kind="ExternalInput").ap()
    wt_d = nc.dram_tensor("wt", [D, N], BF16, kind="ExternalInput").ap()
    w_d = nc.dram_tensor("w", [D, CS], BF16, kind="ExternalInput").ap()
    iota_d = nc.dram_tensor("iota", [128, CW], F16, kind="ExternalInput").ap()
    tcol_d = nc.dram_tensor("tcol", [128, MI, NW], F32, kind="ExternalInput").ap()
    out_d = nc.dram_tensor("out", [N, CS], OUT_DT, kind="ExternalOutput").ap()
    scratch_d = nc.dram_tensor("scratch", [2 * N], F32).ap()

    with tile.TileContext(nc) as tc, ExitStack() as ctx:
        consts = ctx.enter_context(tc.tile_pool(name="consts", bufs=1))
        prep = ctx.enter_context(tc.tile_pool(name="prep", bufs=1))
        small = ctx.enter_context(tc.tile_pool(name="small", bufs=1))
        wpool = ctx.enter_context(tc.tile_pool(name="wpool", bufs=3))
        corrpool = ctx.enter_context(tc.tile_pool(name="corrpool", bufs=4))
        outpool = ctx.enter_context(tc.tile_pool(name="outpool", bufs=3))
        pspool = ctx.enter_context(tc.tile_pool(name="pspool", bufs=6, space="PSUM"))
        cspool = ctx.enter_context(tc.tile_pool(name="cspool", bufs=2, space="PSUM"))

        # ---- resident constants ------------------------------------------------
        xt_sb = consts.tile([128, KI, N], BF16)
        nc.sync.dma_start(out=xt_sb[:], in_=xt_d.rearrange("(k p) n -> p k n", p=128))
        wt_sb = consts.tile([128, KI, N], BF16)
        nc.sync.dma_start(out=wt_sb[:], in_=wt_d.rearrange("(k p) n -> p k n", p=128))
        iota_sb = consts.tile([128, CW], F16)
        nc.sync.dma_start(out=iota_sb[:], in_=iota_d[:])
        tcol_sb = consts.tile([128, MI, NW], F32)
        nc.sync.dma_start(out=tcol_sb[:], in_=tcol_d[:])
        ones_bf = consts.tile([128, 1], BF16)
        nc.vector.memset(ones_bf[:], 1.0)

        # ---- c_t / margin path (tiny) ------------------------------------------
        # column sums of x*wt_hat and x^2 over D -> two [1, N] rows in PSUM
        prod = prep.tile([128, KI, N], BF16)
        xsq = prep.tile([128, KI, N], BF16)
        nc.vector.tensor_mul(prod[:], xt_sb[:], wt_sb[:])
        nc.scalar.square(xsq[:], xt_sb[:])

        pk = prep.tile([1, 2, N], F32)
        for j, src_t in enumerate((prod, xsq)):
            cs_ps = cspool.tile([1, N], F32, tag="cs", name=f"prep_cs_{j}")
            for k in range(KI):
                nc.tensor.matmul(
                    cs_ps[:], ones_bf[:], src_t[:, k, :], start=k == 0, stop=k == KI - 1
                )
            nc.vector.tensor_copy(pk[:, j, :], cs_ps[:])
        # reshape [1, 2, 512] -> [128, 2, 4]: row n = mi*128+p -> (p, mi)
        nc.sync.dma_start(out=scratch_d[:], in_=pk[:])
        vecs = consts.tile([128, 2, MI], F32)
        nc.sync.dma_start(
            out=vecs[:], in_=scratch_d.rearrange("(j m p) -> p j m", p=128, j=2)
        )
        crow_t = vecs[:, 0, :]
        xnsq_t = vecs[:, 1, :]

        xn = small.tile([128, MI], F32)
        nc.scalar.sqrt(xn[:], xnsq_t)
        invxn = small.tile([128, MI], F32)
        nc.vector.reciprocal(invxn[:], xn[:])

        ct = small.tile([128, MI], F32)
        nc.vector.tensor_mul(ct[:], crow_t, invxn[:])
        nc.vector.tensor_scalar_min(ct[:], ct[:], 1.0)
        nc.vector.tensor_scalar_max(ct[:], ct[:], -1.0)

        c2 = small.tile([128, MI], F32)
        nc.vector.tensor_mul(c2[:], ct[:], ct[:])
        cosm = small.tile([128, MI], F32)
        nc.vector.tensor_mul(cosm[:], c2[:], c2[:])  # c^4
        nc.vector.tensor_sub(cosm[:], cosm[:], c2[:])  # c^4 - c^2
        nc.vector.tensor_scalar(cosm[:], cosm[:], 8.0, 1.0, op0=ALU.mult, op1=ALU.add)

        k1 = small.tile([128, MI], F32)
        nc.vector.tensor_single_scalar(k1[:], ct[:], T1, ALU.is_le)
        k2 = small.tile([128, MI], F32)
        nc.vector.tensor_single_scalar(k2[:], ct[:], T2, ALU.is_le)
        k3 = small.tile([128, MI], F32)
        nc.vector.tensor_single_scalar(k3[:], ct[:], T3, ALU.is_le)

        ksum = small.tile([128, MI], F32)
        nc.vector.tensor_add(ksum[:], k1[:], k2[:])
        nc.vector.tensor_add(ksum[:], ksum[:], k3[:])
        sgn = small.tile([128, MI], F32)
        nc.vector.tensor_sub(sgn[:], k1[:], k2[:])
        nc.vector.tensor_add(sgn[:], sgn[:], k3[:])  # k mod 2
        nc.vector.tensor_scalar(sgn[:], sgn[:], -2.0, 1.0, op0=ALU.mult, op1=ALU.add)

        phi = small.tile([128, MI], F32)
        nc.vector.tensor_mul(phi[:], sgn[:], cosm[:])
        nc.vector.tensor_scalar(ksum[:], ksum[:], 2.0, 0.0, op0=ALU.mult, op1=ALU.add)
        nc.vector.tensor_sub(phi[:], phi[:], ksum[:])  # phi = sign*cos_m - 2k

        addn_t = consts.tile([128, MI], F32)
        nc.vector.tensor_sub(addn_t[:], phi[:], ct[:])
        nc.vector.tensor_mul(addn_t[:], addn_t[:], xn[:])
        nc.vector.tensor_scalar(addn_t[:], addn_t[:], INV1PL, 0.0, op0=ALU.mult, op1=ALU.add)

        # ---- main loop over the class dimension --------------------------------
        w_r = w_d.rearrange("(k p) c -> p k c", p=128)
        for ci2 in range(NW):
            c_lo = ci2 * CW
            cw = min(CW, CS - c_lo)
            nh = cw // CTILE

            w_sb = wpool.tile([128, KI * cw], BF16, tag="w", name=f"w_{ci2}")
            nc.sync.dma_start(
                out=w_sb[:].rearrange("p (k c) -> p k c", k=KI),
                in_=w_r[:, :, c_lo : c_lo + cw],
            )

            for mi in range(MI):
                corr = corrpool.tile([128, CW], BF16, tag="corr", name=f"c_{ci2}_{mi}")
                nc.vector.tensor_scalar(
                    corr[:, :cw],
                    iota_sb[:, :cw],
                    tcol_sb[:, mi : mi + 1, ci2 : ci2 + 1],
                    addn_t[:, mi : mi + 1],
                    op0=ALU.is_equal,
                    op1=ALU.mult,
                )
                out_sb = outpool.tile(
                    [128, CW], OUT_DT, tag=f"out{mi}", name=f"o_{ci2}_{mi}"
                )
                for h in range(nh):
                    hs = slice(h * CTILE, (h + 1) * CTILE)
                    ps = pspool.tile([128, CTILE], F32, tag="ps", name=f"ps_{ci2}_{h}_{mi}")
                    for k in range(KI):
                        nc.tensor.matmul(
                            ps[:],
                            xt_sb[:, k, mi * 128 : (mi + 1) * 128],
                            w_sb[:, k * cw + h * CTILE : k * cw + (h + 1) * CTILE],
                            start=k == 0,
                            stop=k == KI - 1,
                        )
                    nc.vector.tensor_add(out_sb[:, hs], ps[:], corr[:, hs])
                nc.sync.dma_start(
                    out=out_d[mi * 128 : (mi + 1) * 128, c_lo : c_lo + cw],
                    in_=out_sb[:, :cw],
                )

    nc.compile()
    return nc


def _get_nc():
    if "nc" not in _CACHE:
        _CACHE["nc"] = _build()
    return _CACHE["nc"]


def _prep_inputs(x, target, weight):
    x = np.asarray(x, dtype=np.float32)
    target = np.asarray(target).astype(np.int64)
    weight = np.asarray(weight, dtype=np.float32)

    # normalize columns in f32, exactly as the reference does, then cast bf16
    w_hat = weight / np.linalg.norm(weight, axis=0, keepdims=True)

    xt_bf = np.ascontiguousarray(x.T).astype(ml_dtypes.bfloat16)
    wt_bf = np.ascontiguousarray(w_hat[:, target]).astype(ml_dtypes.bfloat16)
    iota = np.broadcast_to(np.arange(CW, dtype=np.float16), (128, CW)).copy()

    in_maps = []
    for m in range(NCORES):
        w_bf = np.ascontiguousarray(w_hat[:, m * CS : (m + 1) * CS]).astype(
            ml_dtypes.bfloat16
        )
        # tcol[p, mi, ci2] = target[mi*128+p] - m*CS - ci2*CW, clamped to the
        # sentinel -1 when outside [0, CW) (never matches iota)
        tloc = (target - m * CS).reshape(MI, 128).T  # [128, MI] int64
        tcol = (
            tloc[:, :, None] - (np.arange(NW, dtype=np.int64) * CW)[None, None, :]
        )
        tcol = np.where((tcol >= 0) & (tcol < CW), tcol, -1)
        in_maps.append(
            {
                "xt": xt_bf,
                "wt": wt_bf,
                "w": w_bf,
                "iota": iota,
                "tcol": np.ascontiguousarray(tcol.astype(np.float32)),
            }
        )
    return in_maps


def kernel(x, target, weight, _trace=False, _trace_kwargs=None):
    nc = _get_nc()
    in_maps = _prep_inputs(x, target, weight)
    res = run_bass_kernel_spmd(
        nc,
        in_maps,
        core_ids=list(range(NCORES)),
        trace=_trace,
        **(_trace_kwargs or {}),
    )
    out = np.concatenate(
        [res.results[i]["out"].astype(np.float32) for i in range(NCORES)], axis=1
    )
    if _trace:
        _CACHE["last_result"] = res
    return out


if __name__ == "__main__":
    rng = np.random.default_rng(0)
    x = rng.standard_normal((N, D), dtype=np.float32)
    target = rng.integers(0, C, size=N)
    weight = rng.standard_normal((D, C), dtype=np.float32)
    out = kernel(x, target, weight)
    print("out", out.shape, out.dtype, float(np.abs(out).max()))
